# revision 2
# baseline (speedup 1.0000x reference)
"""LogTransform v2: out = U diag(log(max(s,1e-4))) U^T for 8192 SPD 64x64.

Algorithm (on-device): Y = (X + cI)^-1 via 5 tuned Newton-Schulz steps
(first 3 in bf16, last 2 fp32), z = mu*Y - nu*I, then a degree-24
polynomial in z evaluated as nested Chebyshev (Paterson-Stockmeyer):
P = sum_{j<5} D_j(z) T_j(W), W = T_5(z), Clenshaw over j. Block
combines D_j run as PSUM-preload chains on DVE (j=4,3,2) and as
float32r scaled-identity matmuls on the PE at N=256 (j=1,0, the
b-subtract terms, and the T_5 combine).

Layout: pairs of 64x64 matrices stacked vertically into [128, 64]
columns, 4 pairs per [128, 256] group tile (matmul rhs N=64 per pair);
block-diagonal [128, 512] copies maintained only for matmul lhsT with
persistent zero off-blocks. 8-core data parallel over the batch.
"""
import numpy as np

import concourse.bass as bass
from concourse import bacc
import concourse.tile as tile
from concourse import mybir
from contextlib import ExitStack

F32 = mybir.dt.float32
R32 = mybir.dt.float32r
F16 = mybir.dt.float16
MUL = mybir.AluOpType.mult
ADD = mybir.AluOpType.add
SUB = mybir.AluOpType.subtract

# ---- problem geometry ----
B, D = 8192, 64
N_CORES = 8
PER_CORE = B // N_CORES          # 1024 matrices
N_PAIRS = PER_CORE // 2          # 512 pairs
PPG = 8                          # pairs per group
N_GROUPS = N_PAIRS // PPG        # 128 groups
GW = 64 * PPG                    # 256 stacked free width
BW = 128 * PPG                   # 512 block-diag width

# ---- tuned constants (offline fit, scalar relfro 0.0048 w/ bf16 NS) ----
C_SHIFT = 0.02
E0 = 1.2365239436619722
E1 = -0.19350922436024606
NS = [(3.814252, 1.907126), (3.398118, 1.699059), (2.646699, 1.323349),
      (2.110322, 1.055161), (2.003047, 1.001524)]
N_NS = len(NS)
N_BF = 4                         # first N_BF NS steps in fp16
MU = 0.04012578299014515
NU = 1.0062991810031625
# gamma[j][i]: P = sum_j D_j(z) T_j(W), D_j = sum_i gamma[j][i] z^i
GAM = [[-3.9108196829030764, -1.307614709457099,
        -0.05735564591668078, -3.0006572675071603, 0.0],
       [-0.12782971587319145, 0.0, 0.0,
        -0.03599215886595403, -0.8794252958452543],
       [0.0, 0.0, -0.11248237618057898,
        -0.18780449648950048, 0.2263929261829175],
       [0.0, 0.0, 0.0, 0.06371564226731165, -0.01658310581958522],
       [0.0, -0.04602259394239294, -0.1185018640804337,
        0.0990333671439928, 0.23375173820291587]]

# const [128,256] tile indices in k dram param
KI_E0 = 0                        # E0 * I
KI_AL = 1                        # 1..5: alpha_k * I
KI_NU = 6                        # nu * I
KI_G = 7                         # 7..10: gamma[4..1][0]*I ; 11: 2*gamma[0][0]*I
N_CONST = 12

# scaled-identity [128,128] lhsT consts in ki dram param
KJ_I = 0                         # I
KJ_NEG = 1                       # -I
KJ_NEG2 = 2                      # -2 I
KJ_G22 = 3                       # 3..5: gamma[2][2..4] * I
KJ_G13 = 6                       # 6..7: gamma[1][3..4] * I
KJ_G01 = 8                       # 8..10: 2*gamma[0][1..3] * I
N_KI = 11


def _consts() -> np.ndarray:
    eye64 = np.eye(64, dtype=np.float32)
    ist = np.concatenate([np.concatenate([eye64, eye64], 0)] * PPG, 1)
    ks = [E0 * ist]
    for al, _ in NS:
        ks.append(al * ist)
    ks.append(NU * ist)
    for j in (4, 3, 2, 1):
        ks.append(GAM[j][0] * ist)
    ks.append(2.0 * GAM[0][0] * ist)
    return np.stack(ks).astype(np.float32)


def _iconsts() -> np.ndarray:
    eye = np.eye(128, dtype=np.float32)
    ks = [eye, -eye, -2.0 * eye]
    for i in (2, 3, 4):
        ks.append(GAM[2][i] * eye)
    for i in (3, 4):
        ks.append(GAM[1][i] * eye)
    for i in (1, 2, 3):
        ks.append(2.0 * GAM[0][i] * eye)
    return np.stack(ks).astype(np.float32)


def _re3(ap):
    return ap.rearrange("p (q c) -> p q c", q=PPG)


def _re3b(ap):
    return ap.rearrange("p (q c) -> p q c", q=PPG)


DBG_TAP = None                   # stage name to stream to the dbg output
SKIP_NS = False                  # profiling: skip NS iteration
SKIP_CHEB = False                # profiling: skip chebyshev stage
WBUFS = 2                        # SBUF work pool depth (groups in flight)
PBUFS = 2                        # PSUM pool depth per tag


def build_nc(n_groups: int = N_GROUPS) -> bass.Bass:
    nc = bacc.Bacc("TRN2", target_bir_lowering=False)
    a_in = nc.declare_dram_parameter("a", [n_groups, 128, GW], F32,
                                     isOutput=False)
    abd_in = nc.declare_dram_parameter("abd", [n_groups, 128, BW], F32,
                                       isOutput=False)
    k_in = nc.declare_dram_parameter("k", [N_CONST, 128, GW], F32,
                                     isOutput=False)
    ki_in = nc.declare_dram_parameter("ki", [N_KI, 128, 128], F32,
                                      isOutput=False)
    o_out = nc.declare_dram_parameter("o", [n_groups, 128, GW], F32,
                                      isOutput=True)
    dbg_out = None
    if DBG_TAP is not None:
        dbg_out = nc.declare_dram_parameter("dbg", [n_groups, 128, GW], F32,
                                            isOutput=True)

    with ExitStack() as ctx:
        tc = ctx.enter_context(tile.TileContext(nc))
        kpool = ctx.enter_context(tc.tile_pool(name="kpool", bufs=1))
        wpool = ctx.enter_context(tc.tile_pool(name="wpool", bufs=WBUFS))
        ppool = ctx.enter_context(tc.tile_pool(name="ppool", bufs=PBUFS,
                                               space="PSUM"))

        K = []
        for i in range(N_CONST):
            kt = kpool.tile([128, GW], F32, name=f"k{i}", tag=f"k{i}")
            nc.sync.dma_start(out=kt[:], in_=k_in[i])
            K.append(kt)
        KI = []
        for i in range(N_KI):
            kt = kpool.tile([128, 128], F32, name=f"kiw{i}", tag=f"kiw{i}")
            nc.sync.dma_start(out=kt[:], in_=ki_in[i])
            kr = kpool.tile([128, 128], R32, name=f"ki{i}", tag=f"ki{i}")
            nc.scalar.copy(out=kr[:], in_=kt[:])
            KI.append(kr)
        KGR = {}
        for i in (KI_G + 3, KI_G + 4):
            kr = kpool.tile([128, GW], R32, name=f"kgr{i}", tag=f"kgr{i}")
            nc.scalar.copy(out=kr[:], in_=K[i][:])
            KGR[i] = kr

        def persist(name, n, dt):
            ts = []
            for i in range(n):
                t = kpool.tile([128, BW], dt, name=f"{name}{i}",
                               tag=f"{name}{i}")
                nc.vector.memset(t[:], 0)
                ts.append(t)
            return ts

        BBD16 = persist("bbd16", 5, F16)
        BBD32 = persist("bbd32", 2, F32)
        ZBD = persist("zbd", 2, F32)
        WBD = persist("wbd", 3, F32)

        def conv_bd(eng, dst, src, scale=1.0):
            """2 strided copies: stacked [128,256] -> block-diag [128,512]."""
            s3, d3 = _re3(src), _re3b(dst)
            if eng is nc.scalar:
                nc.scalar.mul(out=d3[0:64, :, 0:64], in_=s3[0:64], mul=scale)
                nc.scalar.mul(out=d3[64:128, :, 64:128], in_=s3[64:128],
                              mul=scale)
            else:
                eng.tensor_scalar_mul(d3[0:64, :, 0:64], s3[0:64], scale)
                eng.tensor_scalar_mul(d3[64:128, :, 64:128], s3[64:128],
                                      scale)

        def mm_pass(psum, lhsT, rhs, start=True):
            for q in range(PPG):
                nc.tensor.matmul(psum[:, q * 64:(q + 1) * 64],
                                 lhsT=lhsT[:, q * 128:(q + 1) * 128],
                                 rhs=rhs[:, q * 64:(q + 1) * 64],
                                 start=start, stop=True,
                                 skip_group_check=not start)

        def idmm(psum, kj, rhs, start=False):
            """psum += const_diag(kj) @ rhs as one N=256 float32r matmul."""
            nc.tensor.matmul(psum[:, :], lhsT=KI[kj][:],
                             rhs=rhs[:] if rhs.dtype == R32
                             else rhs[:].bitcast(R32),
                             start=start, stop=True, skip_group_check=True)

        def tap(name, ap, g):
            if DBG_TAP == name:
                dt = wpool.tile([128, GW], F32, tag="dbgt")
                if ap.dtype != F32:
                    nc.vector.tensor_scalar_mul(dt[:], ap, 1.0)
                    nc.sync.dma_start(out=dbg_out[g], in_=dt[:])
                else:
                    nc.sync.dma_start(out=dbg_out[g], in_=ap)

        S = [dict() for _ in range(n_groups)]

        def ns_step(g, ks, v):
            st = S[g]
            lo = ks < N_BF
            up = ppool.tile([128, GW], F32, tag="u", bufs=2)
            mm_pass(up, st['abdh'] if lo else st['abd'], v)
            bst = wpool.tile([128, GW], F16 if lo else F32,
                             tag="bst16" if lo else "bst32",
                             bufs=4 if lo else 2)
            nc.vector.scalar_tensor_tensor(bst[:], up[:], -NS[ks][1],
                                           K[KI_AL + ks][:], MUL, ADD)
            bbd = BBD16[(4 * g + ks) % len(BBD16)] if lo else \
                BBD32[g % len(BBD32)]
            conv_bd(nc.gpsimd, bbd, bst)
            vp = ppool.tile([128, GW], F32, tag="vp", bufs=2)
            mm_pass(vp, bbd, v)
            nxt_lo = (ks + 1) < N_BF
            v = wpool.tile([128, GW], F16 if nxt_lo else F32,
                           tag="v16" if nxt_lo else "v32",
                           bufs=4 if nxt_lo else 3)
            nc.scalar.copy(out=v[:], in_=vp[:])
            return v

        def m0(g):
            st = S[g]
            ast = wpool.tile([128, GW], F32, tag="ast", bufs=2)
            nc.sync.dma_start(out=ast[:], in_=a_in[g])
            abd = wpool.tile([128, BW], F32, tag="abd", bufs=2)
            nc.sync.dma_start(out=abd[:], in_=abd_in[g])
            abdh = wpool.tile([128, BW], F16, tag="abdh", bufs=2)
            nc.scalar.copy(out=abdh[:], in_=abd[:])
            st['abd'], st['abdh'] = abd, abdh
            v = wpool.tile([128, GW], F16, tag="v16", bufs=4)
            nc.vector.scalar_tensor_tensor(v[:], ast[:], E1, K[KI_E0][:],
                                           MUL, ADD)
            v = ns_step(g, 0, v)
            st['v'] = ns_step(g, 1, v)

        def m1(g):
            st = S[g]
            v = st['v']
            for ks in range(2, N_NS):
                v = ns_step(g, ks, v)
            st['v'] = v

        def m2(g):
            st = S[g]
            zr = wpool.tile([128, GW], F32, tag="zr", bufs=2)
            nc.vector.scalar_tensor_tensor(zr[:], st['v'][:], MU,
                                           K[KI_NU][:], MUL, SUB)
            z = wpool.tile([128, GW], F32, tag="z", bufs=2)
            nc.scalar.copy(out=z[:], in_=zr[:])
            zbd = ZBD[g % len(ZBD)]
            conv_bd(nc.gpsimd, zbd, zr)
            pws = [z]
            for i in (2, 3, 4):
                pp = ppool.tile([128, GW], F32, tag="pw", bufs=2)
                mm_pass(pp, zbd, pws[-1])
                pz = wpool.tile([128, GW], F32, tag=f"z{i}", bufs=2)
                nc.scalar.copy(out=pz[:], in_=pp[:])
                pws.append(pz)
            wps = ppool.tile([128, GW], F32, tag="pw", bufs=2)
            mm_pass(wps, zbd, pws[3])               # z^5
            # W/16 = z^5 - 1.25 z^3 + 0.3125 z (fp32: heavy cancellation)
            wt = wpool.tile([128, GW], F32, tag="wt", bufs=2)
            nc.vector.scalar_tensor_tensor(wt[:], pws[2], -1.25, wps[:],
                                           MUL, ADD)
            nc.vector.scalar_tensor_tensor(wt[:], pws[0], 0.3125, wt[:],
                                           MUL, ADD)
            wbd = WBD[g % len(WBD)]
            conv_bd(nc.gpsimd, wbd, wt, scale=32.0)  # 2W as lhsT
            pr = []
            for i, src_t in enumerate(pws):
                dd = wpool.tile([128, GW], R32, tag=f"zr32_{i}", bufs=2)
                nc.gpsimd.tensor_scalar_mul(dd[:], src_t[:], 1.0)
                pr.append(dd)
            st['pws'], st['pr'], st['wbd'] = pws, pr, wbd

        def m3(g):
            st = S[g]
            pws, pr, wbd = st['pws'], st['pr'], st['wbd']

            def r32dup(src_t, tg):
                dd = wpool.tile([128, GW], R32, tag=tg, bufs=2)
                nc.gpsimd.tensor_scalar_mul(dd[:], src_t[:], 1.0)
                return dd

            # b4 = D_4 = g41 z + g42 z^2 + g43 z^3 + g44 z^4 (DVE)
            b4r = wpool.tile([128, GW], F32, tag="b4r", bufs=2)
            nc.vector.tensor_scalar_mul(b4r[:], pws[0], GAM[4][1])
            nc.vector.scalar_tensor_tensor(b4r[:], pws[1], GAM[4][2],
                                           b4r[:], MUL, ADD)
            nc.vector.scalar_tensor_tensor(b4r[:], pws[2], GAM[4][3],
                                           b4r[:], MUL, ADD)
            nc.vector.scalar_tensor_tensor(b4r[:], pws[3], GAM[4][4],
                                           b4r[:], MUL, ADD)
            b4 = wpool.tile([128, GW], F32, tag="b4", bufs=2)
            nc.scalar.copy(out=b4[:], in_=b4r[:])
            b4d = r32dup(b4r, "b4d")
            # b3 = 2W b4 + D_3 ; D_3 = g33 z^3 + g34 z^4 (DVE preload)
            c3 = ppool.tile([128, GW], F32, tag="cl", bufs=2)
            t3 = wpool.tile([128, GW], F32, tag="ch", bufs=2)
            nc.vector.tensor_scalar_mul(t3[:], pws[2], GAM[3][3])
            nc.vector.scalar_tensor_tensor(c3[:], pws[3], GAM[3][4],
                                           t3[:], MUL, ADD)
            mm_pass(c3, wbd, b4, start=False)
            b3 = wpool.tile([128, GW], F32, tag="b3", bufs=2)
            nc.scalar.copy(out=b3[:], in_=c3[:])
            b3d = r32dup(b3, "b3d")
            # b2 = 2W b3 + D_2 - b4 ; D_2 via PE f32r identity matmuls
            c2 = ppool.tile([128, GW], F32, tag="cl", bufs=2)
            idmm(c2, KJ_G22 + 0, pr[1], start=True)
            idmm(c2, KJ_G22 + 1, pr[2])
            idmm(c2, KJ_G22 + 2, pr[3])
            idmm(c2, KJ_NEG, b4d)
            mm_pass(c2, wbd, b3, start=False)
            b2 = wpool.tile([128, GW], F32, tag="b2", bufs=2)
            nc.scalar.copy(out=b2[:], in_=c2[:])
            b2d = r32dup(b2, "b2d")
            # b1 = 2W b2 + D_1 - b3
            c1 = ppool.tile([128, GW], F32, tag="cl", bufs=2)
            idmm(c1, KJ_I, KGR[KI_G + 3], start=True)   # gamma[1][0] I
            idmm(c1, KJ_G13 + 0, pr[2])
            idmm(c1, KJ_G13 + 1, pr[3])
            idmm(c1, KJ_NEG, b3d)
            mm_pass(c1, wbd, b2, start=False)
            b1 = wpool.tile([128, GW], F32, tag="b1", bufs=2)
            nc.scalar.copy(out=b1[:], in_=c1[:])
            # out = W b1 + D_0 - b2 (x2 in bank, halve on copy-out)
            c0 = ppool.tile([128, GW], F32, tag="cl", bufs=2)
            idmm(c0, KJ_I, KGR[KI_G + 4], start=True)   # 2 gamma[0][0] I
            idmm(c0, KJ_G01 + 0, pr[0])
            idmm(c0, KJ_G01 + 1, pr[1])
            idmm(c0, KJ_G01 + 2, pr[2])
            idmm(c0, KJ_NEG2, b2d)
            mm_pass(c0, wbd, b1, start=False)
            ot = wpool.tile([128, GW], F32, tag="ot", bufs=2)
            nc.scalar.mul(out=ot[:], in_=c0[:], mul=0.5)
            nc.sync.dma_start(out=o_out[g], in_=ot[:])

        # software-pipelined emission: 4 groups in flight, stage-skewed
        ms = [m0, m1, m2, m3]
        for t in range(n_groups + 3):
            for s in range(3, -1, -1):
                g = t - s
                if 0 <= g < n_groups:
                    ms[s](g)

    nc.compile()
    return nc


# ---------------- host side ----------------

def _pack_core(shard: np.ndarray, n_groups: int):
    n = shard.shape[0]
    sh = shard.copy()
    idx = np.arange(D)
    sh[:, idx, idx] += np.float32(C_SHIFT)
    p = sh.reshape(n_groups, PPG, 2, D, D)
    ast = np.ascontiguousarray(
        p.transpose(0, 2, 3, 1, 4).reshape(n_groups, 128, GW))
    blocks = np.zeros((n // 2, 128, 128), dtype=np.float32)
    pp = sh.reshape(n // 2, 2, D, D)
    blocks[:, :D, :D] = pp[:, 0]
    blocks[:, D:, D:] = pp[:, 1]
    abd = np.ascontiguousarray(
        blocks.reshape(n_groups, PPG, 128, 128)
        .transpose(0, 2, 1, 3).reshape(n_groups, 128, BW))
    return ast, abd


def _unpack_core(o: np.ndarray, n_groups: int) -> np.ndarray:
    p = o.reshape(n_groups, 2, D, PPG, D).transpose(0, 3, 1, 2, 4)
    return np.ascontiguousarray(p.reshape(n_groups * PPG * 2, D, D))


_NC_CACHE = {}
_JIT_CACHE = {}


def _get_nc(n_groups):
    if n_groups not in _NC_CACHE:
        _NC_CACHE[n_groups] = build_nc(n_groups)
    return _NC_CACHE[n_groups]


def _get_runner(n_groups, n_cores):
    """Build (once) a cached jitted SPMD runner mirroring
    bass2jax.run_bass_via_pjrt, so repeat calls skip retrace/recompile."""
    key = (n_groups, n_cores)
    if key in _JIT_CACHE:
        return _JIT_CACHE[key]
    import jax
    from jax.sharding import Mesh, PartitionSpec
    from jax.experimental.shard_map import shard_map
    from concourse import bass2jax
    nc = _get_nc(n_groups)
    bass2jax.install_neuronx_cc_hook()

    in_names = []
    out_names = []
    out_avals = []
    zero_outs = []
    partition_name = (nc.partition_id_tensor.name
                      if nc.partition_id_tensor else None)
    for alloc in nc.m.functions[0].allocations:
        if not isinstance(alloc, mybir.MemoryLocationSet):
            continue
        name = alloc.memorylocations[0].name
        if alloc.kind == "ExternalInput":
            if name != partition_name:
                in_names.append(name)
        elif alloc.kind == "ExternalOutput":
            shape = tuple(alloc.tensor_shape)
            dtype = mybir.dt.np(alloc.dtype)
            out_names.append(name)
            out_avals.append(jax.core.ShapedArray(shape, dtype))
            zero_outs.append(np.zeros(shape, dtype))
    n_params = len(in_names)
    all_in = list(in_names) + list(out_names)
    if partition_name is not None:
        all_in.append(partition_name)

    def _body(*args):
        operands = list(args)
        if partition_name is not None:
            operands.append(bass2jax.partition_id_tensor())
        outs = bass2jax._bass_exec_p.bind(
            *operands,
            out_avals=tuple(out_avals),
            in_names=tuple(all_in),
            out_names=tuple(out_names),
            lowering_input_output_aliases=(),
            sim_require_finite=True,
            sim_require_nnan=True,
            nc=nc,
        )
        return tuple(outs)

    devices = jax.devices()[:n_cores]
    mesh = Mesh(np.asarray(devices), ("core",))
    in_specs = (PartitionSpec("core"),) * (n_params + len(out_names))
    out_specs = (PartitionSpec("core"),) * len(out_names)
    sharded = jax.jit(shard_map(_body, mesh=mesh, in_specs=in_specs,
                                out_specs=out_specs, check_rep=False),
                      keep_unused=True)
    runner = (sharded, in_names, out_names, zero_outs)
    _JIT_CACHE[key] = runner
    return runner


def _execute(in_maps, n_groups, n_cores):
    sharded, in_names, out_names, zero_outs = _get_runner(n_groups, n_cores)
    concat_in = [np.concatenate([np.asarray(in_maps[c][nm])
                                 for c in range(n_cores)], axis=0)
                 for nm in in_names]
    concat_zeros = [np.zeros((n_cores * z.shape[0], *z.shape[1:]), z.dtype)
                    for z in zero_outs]
    outs = sharded(*concat_in, *concat_zeros)
    res = []
    for c in range(n_cores):
        res.append({nm: np.asarray(outs[i]).reshape(
            n_cores, *zero_outs[i].shape)[c]
            for i, nm in enumerate(out_names)})
    return res


def run(x: np.ndarray, n_groups: int = N_GROUPS):
    x = np.ascontiguousarray(x, dtype=np.float32)
    npc = n_groups * PPG * 2
    consts = _consts()
    iconsts = _iconsts()
    in_maps = []
    for c in range(N_CORES):
        ast, abd = _pack_core(x[c * npc:(c + 1) * npc], n_groups)
        in_maps.append({"a": ast, "abd": abd, "k": consts, "ki": iconsts})
    res = _execute(in_maps, n_groups, N_CORES)
    outs = [_unpack_core(res[c]["o"], n_groups) for c in range(N_CORES)]
    return np.concatenate(outs, axis=0)


def kernel(x: np.ndarray) -> np.ndarray:
    return run(x)


def bench(x: np.ndarray, iters: int = 10, n_groups: int = N_GROUPS):
    """Time device execution with device-resident inputs (excludes host
    packing and host<->device transfer). Returns per-call seconds list."""
    import time
    import jax
    x = np.ascontiguousarray(x, dtype=np.float32)
    npc = n_groups * PPG * 2
    consts = _consts()
    iconsts = _iconsts()
    in_maps = []
    for c in range(N_CORES):
        ast, abd = _pack_core(x[c * npc:(c + 1) * npc], n_groups)
        in_maps.append({"a": ast, "abd": abd, "k": consts, "ki": iconsts})
    sharded, in_names, out_names, zero_outs = _get_runner(n_groups, N_CORES)
    concat_in = [np.concatenate([np.asarray(in_maps[c][nm])
                                 for c in range(N_CORES)], axis=0)
                 for nm in in_names]
    concat_zeros = [np.zeros((N_CORES * z.shape[0], *z.shape[1:]), z.dtype)
                    for z in zero_outs]
    dev_in = [jax.device_put(a) for a in concat_in]
    dev_zero = [jax.device_put(a) for a in concat_zeros]
    outs = sharded(*dev_in, *dev_zero)          # warm + compile
    jax.block_until_ready(outs)
    times = []
    for _ in range(iters):
        t0 = time.perf_counter()
        outs = sharded(*dev_in, *dev_zero)
        jax.block_until_ready(outs)
        times.append(time.perf_counter() - t0)
    return times
